# revision 14
# baseline (speedup 1.0000x reference)
"""TRN2 Bass kernel for the quantized 4-layer MLP (dense_mlp, 8 cores).

Strategy (v3):
  - Data-parallel over batch: each of the 8 cores gets 1024 of 8192 rows.
  - All weights quantized to integer LEVELS on host (bit-exact replica of
    the reference wquant: round(W/s) with RTNE), shipped as fp16 (layer-1
    hi) / fp8e4 (everything else). No on-device weight quantization.
  - All DRAM operands pre-arranged on host so each SBUF partition's data
    is contiguous in DRAM (fat DMA descriptors; the naive
    "(kt p) n -> p kt n" rearrange yields 128B lines at ~13GB/s).
  - Layer 1 x split: hi = fp16(x) [16 matmuls/tile], lo = fp8(r*2^9) with
    stationary levels*2^-9 (exact in fp8e4 subnormals) via DoubleRow
    [8 matmuls/tile]. hi/lo accumulate in separate PSUM banks (mixing
    perf modes in one accumulation group is broken on HW); the epilogue
    DVE scalar_tensor_tensor folds lo in: tmp = lo_psum*alpha + act(hi).
    Combined x precision ~2^-15; measured end-to-end rel err 1.04e-2
    (gate 2e-2). L1 runs in 2-nt groups (hi x4 then lo x4) to halve the
    fp16<->DoubleRow mode-switch penalty (~0.4us per switch).
  - Layers 2-4: fp8e4 DoubleRow matmuls over integer levels - bit-exact,
    2x tensor-engine throughput.
  - BN + QuantReLU epilogue fused: ACT per-feature affine, DVE round
    (+C/-C trick), DVE clip(15,0) with fp8 output cast.
  - Total matmuls/core: 1536 (L1) + 1024 (L2) + 1024 (L3) + 256 (L4)
    = 3840 @ ~216ns issue rate -> ~830us floor.
"""

import numpy as np
import ml_dtypes

B, D_IN, H, C_OUT = 8192, 2048, 4096, 1000
NCORES = 8
BC = B // NCORES            # 1024 batch rows per core
N4P = 1024                  # padded final output feature dim (1000 -> 1024)
C_ROUND = float(1.5 * 2 ** 23)
EPS = 1e-5
LO_SC = 512.0               # 2^9 residual scale for the fp8 lo pass
P = 128
KTX = D_IN // P             # 16
KT = H // P                 # 32
NT1 = H // P                # 32
NT4 = N4P // P              # 8

_CACHE = {}


def _build_nc():
    import concourse.bass as bass  # noqa: F401
    from concourse import bacc
    import concourse.mybir as mybir
    import concourse.tile as tile

    dt = mybir.dt
    AF = mybir.ActivationFunctionType
    ALU = mybir.AluOpType
    DR = mybir.MatmulPerfMode.DoubleRow

    nc = bacc.Bacc("TRN2", target_bir_lowering=False)

    # ---- DRAM I/O (all pre-arranged: partition-contiguous) ----
    xh_d = nc.dram_tensor("xh", [P, KTX * BC], dt.float16, kind="ExternalInput")
    xl_d = nc.dram_tensor("xl", [P, KTX * BC], dt.float8e4, kind="ExternalInput")
    w1h_d = nc.dram_tensor("w1h", [NT1, P, KTX * P], dt.float16, kind="ExternalInput")
    w1l_d = nc.dram_tensor("w1l", [NT1, P, KTX * P], dt.float8e4, kind="ExternalInput")
    w2_d = nc.dram_tensor("w2", [NT1, P, KT * P], dt.float8e4, kind="ExternalInput")
    w3_d = nc.dram_tensor("w3", [NT1, P, KT * P], dt.float8e4, kind="ExternalInput")
    w4_d = nc.dram_tensor("w4", [NT4, P, KT * P], dt.float8e4, kind="ExternalInput")
    ab1 = nc.dram_tensor("ab1", [P, NT1 * 2], dt.float32, kind="ExternalInput")
    ab2 = nc.dram_tensor("ab2", [P, NT1 * 2], dt.float32, kind="ExternalInput")
    ab3 = nc.dram_tensor("ab3", [P, NT1 * 2], dt.float32, kind="ExternalInput")
    ab4 = nc.dram_tensor("ab4", [P, NT4 * 2], dt.float32, kind="ExternalInput")
    out_t = nc.dram_tensor("out_t", [N4P, BC], dt.float32, kind="ExternalOutput")

    with tile.TileContext(nc) as tc:
        ppool_cm = tc.tile_pool(name="psum", bufs=3, space="PSUM")
        ppool = ppool_cm.__enter__()
        plpool_cm = tc.tile_pool(name="psuml", bufs=5, space="PSUM")
        plpool = plpool_cm.__enter__()

        abt_cm = tc.tile_pool(name="abtp", bufs=1)
        abt_pool = abt_cm.__enter__()
        ncbias = abt_pool.tile([P, 1], dt.float32, name="ncbias")
        nc.vector.memset(ncbias[:], -C_ROUND)
        tmp_cm = tc.tile_pool(name="tmpp", bufs=4)
        tmp_pool = tmp_cm.__enter__()

        apool12_cm = tc.tile_pool(name="acts12", bufs=1)
        apool12 = apool12_cm.__enter__()
        A1 = apool12.tile([P, KT, BC], dt.float8e4, name="A1")

        def epilogue(psum, abt, nt, b0, a_out, lo_psum=None):
            tmp = tmp_pool.tile([P, 512], dt.float32, name="tmp", tag="tmp")
            if a_out is not None:
                nc.scalar.activation(
                    tmp[:], psum[:], AF.Identity,
                    bias=abt[:, 2 * nt + 1:2 * nt + 2], scale=abt[:, 2 * nt:2 * nt + 1])
                if lo_psum is not None:
                    nc.vector.scalar_tensor_tensor(
                        tmp[:], lo_psum[:], abt[:, 2 * nt:2 * nt + 1], tmp[:],
                        ALU.mult, ALU.add)
                nc.vector.tensor_scalar(tmp[:], tmp[:], C_ROUND, C_ROUND,
                                        ALU.add, ALU.subtract)
                nc.vector.tensor_scalar(a_out[:, nt, b0:b0 + 512], tmp[:],
                                        15.0, 0.0, ALU.min, ALU.max)
            else:
                ost = tmp_pool.tile([P, 512], dt.float32, name="ost", tag="ost")
                nc.scalar.activation(
                    ost[:], psum[:], AF.Identity,
                    bias=abt[:, 2 * nt + 1:2 * nt + 2], scale=abt[:, 2 * nt:2 * nt + 1])
                n0 = nt * P
                nc.gpsimd.dma_start(out_t[n0:n0 + P, b0:b0 + 512], ost[:])

        # ---- layer 1: fp16 hi + fp8 DR lo, separate psums ----
        xt_pool_cm = tc.tile_pool(name="xtp", bufs=1)
        xt_pool = xt_pool_cm.__enter__()
        xh_t = xt_pool.tile([P, KTX, BC], dt.float16, name="xh_t")
        xl_t = xt_pool.tile([P, KTX, BC], dt.float8e4, name="xl_t")

        w1_cm = tc.tile_pool(name="w1p", bufs=4)
        w1_pool = w1_cm.__enter__()
        w1l_cm = tc.tile_pool(name="w1lp", bufs=9)
        w1l_pool = w1l_cm.__enter__()
        l1tmp_cm = tc.tile_pool(name="l1tmp", bufs=16)
        l1tmp_pool = l1tmp_cm.__enter__()

        abt1 = abt_pool.tile([P, NT1 * 2], dt.float32, name="abt1")

        def w1_fetch(nt):
            w1h_t = w1_pool.tile([P, KTX, P], dt.float16, name="w1h_t", tag="wh")
            w1l_t = w1l_pool.tile([P, KTX, P], dt.float8e4, name="w1l_t", tag="wl")
            nc.sync.dma_start(
                w1h_t[:], w1h_d[nt].rearrange("p (kt n) -> p kt n", n=P))
            nc.sync.dma_start(
                w1l_t[:], w1l_d[nt].rearrange("p (kt n) -> p kt n", n=P))
            return w1h_t, w1l_t

        # DMA order: xh chunk 0 leads the gpsimd queue while the nt0 hi
        # weights lead the sync queue (parallel critical path); remaining
        # xh chunks alternate queues; xl (needed only ~60us in) follows.
        nc.gpsimd.dma_start(xh_t[:, 0:2, :],
                            xh_d[:, 0:2 * BC].rearrange(
                                "p (kt b) -> p kt b", b=BC))
        w1h_t0 = w1_pool.tile([P, KTX, P], dt.float16, name="w1h_t", tag="wh")
        nc.sync.dma_start(
            w1h_t0[:], w1h_d[0].rearrange("p (kt n) -> p kt n", n=P))
        for c0 in range(2, KTX, 2):
            q = nc.sync if (c0 // 2) % 2 == 0 else nc.gpsimd
            q.dma_start(xh_t[:, c0:c0 + 2, :],
                        xh_d[:, c0 * BC:(c0 + 2) * BC].rearrange(
                            "p (kt b) -> p kt b", b=BC))
        w1l_t0 = w1l_pool.tile([P, KTX, P], dt.float8e4, name="w1l_t", tag="wl")
        nc.sync.dma_start(
            w1l_t0[:], w1l_d[0].rearrange("p (kt n) -> p kt n", n=P))
        w1_tiles = {0: (w1h_t0, w1l_t0)}
        nc.gpsimd.dma_start(abt1[:], ab1[:])
        for c0 in range(0, KTX, 4):
            q = nc.gpsimd if (c0 // 4) % 2 == 0 else nc.sync
            q.dma_start(xl_t[:, c0:c0 + 4, :],
                        xl_d[:, c0 * BC:(c0 + 4) * BC].rearrange(
                            "p (kt b) -> p kt b", b=BC))

        # process nt in groups of 8: hi passes (ACT spills psum to SBUF
        # right away), then lo passes + combine epilogue. 2 fp16<->DR
        # mode switches per group instead of per nt.
        G1 = 8
        for ntp in range(0, NT1, G1):
            grp = range(ntp, ntp + G1)
            tasks = [(nt, b0) for nt in grp for b0 in (0, 512)]
            tmps = {}
            for nt in grp:
                if nt + 1 not in w1_tiles and nt + 1 < NT1:
                    w1_tiles[nt + 1] = w1_fetch(nt + 1)
                # kt-major over both batch halves: each x chunk feeds 2x
                # the matmul work while later chunks stream in, and the
                # two matmuls of a kt share the same stationary tile.
                phs = {b0: ppool.tile([P, 512], dt.float32, name="ps_h",
                                      tag="ph") for b0 in (0, 512)}
                for kt in range(KTX):
                    for b0 in (0, 512):
                        nc.tensor.matmul(
                            phs[b0][:], w1_tiles[nt][0][:, kt, :],
                            xh_t[:, kt, b0:b0 + 512],
                            start=(kt == 0), stop=(kt == KTX - 1))
                for b0 in (0, 512):
                    tmp = l1tmp_pool.tile([P, 512], dt.float32, name="l1t",
                                          tag="l1t")
                    nc.scalar.activation(
                        tmp[:], phs[b0][:], AF.Identity,
                        bias=abt1[:, 2 * nt + 1:2 * nt + 2],
                        scale=abt1[:, 2 * nt:2 * nt + 1])
                    tmps[(nt, b0)] = tmp
            for nt, b0 in tasks:
                pl = plpool.tile([P, 512], dt.float32, name="ps_l", tag="pl")
                for kp in range(KTX // 2):
                    nc.tensor.matmul(
                        pl[:], w1_tiles[nt][1][:, 2 * kp:2 * kp + 2, :],
                        xl_t[:, 2 * kp:2 * kp + 2, b0:b0 + 512],
                        start=(kp == 0), stop=(kp == KTX // 2 - 1),
                        perf_mode=DR)
                tmp = tmps[(nt, b0)]
                nc.vector.scalar_tensor_tensor(
                    tmp[:], pl[:], abt1[:, 2 * nt:2 * nt + 1], tmp[:],
                    ALU.mult, ALU.add)
                # round via +C (fp32 RTNE at integer granularity), clip
                # high on DVE; low clip + -C + fp8 cast ride the scalar
                # engine (Relu) to keep DVE under the lo-phase MM rate.
                nc.vector.tensor_scalar(tmp[:], tmp[:], C_ROUND,
                                        C_ROUND + 15.0, ALU.add, ALU.min)
                nc.scalar.activation(A1[:, nt, b0:b0 + 512], tmp[:],
                                     AF.Relu, bias=ncbias[:], scale=1.0)
            for nt in grp:
                del w1_tiles[nt]

        l1tmp_cm.__exit__(None, None, None)
        w1l_cm.__exit__(None, None, None)
        w1_cm.__exit__(None, None, None)
        xt_pool_cm.__exit__(None, None, None)

        # ---- layers 2-4: fp8 DR with preloaded level weights ----
        apool23_cm = tc.tile_pool(name="acts23", bufs=1)
        apool23 = apool23_cm.__enter__()
        A2 = apool23.tile([P, KT, BC], dt.float8e4, name="A2")
        A3 = apool23.tile([P, KT, BC], dt.float8e4, name="A3")

        wf_cm = tc.tile_pool(name="wfp", bufs=3)
        wf_pool = wf_cm.__enter__()

        def layer_dr(wt, abt, NT, a_in, a_out, idx):
            for nt in range(NT):
                qt = wf_pool.tile([P, KT, P], dt.float8e4, name=f"qt{idx}",
                                  tag="qt")
                nc.sync.dma_start(
                    qt[:], wt[nt].rearrange("p (kt n) -> p kt n", n=P))
                for b0 in (0, 512):
                    psum = ppool.tile([P, 512], dt.float32, name="ps", tag="ph")
                    for kp in range(KT // 2):
                        nc.tensor.matmul(
                            psum[:], qt[:, 2 * kp:2 * kp + 2, :],
                            a_in[:, 2 * kp:2 * kp + 2, b0:b0 + 512],
                            start=(kp == 0), stop=(kp == KT // 2 - 1),
                            perf_mode=DR)
                    epilogue(psum, abt, nt, b0, a_out)

        abt2 = abt_pool.tile([P, NT1 * 2], dt.float32, name="abt2")
        nc.gpsimd.dma_start(abt2[:], ab2[:])
        layer_dr(w2_d, abt2, NT1, A1, A2, 2)
        abt3 = abt_pool.tile([P, NT1 * 2], dt.float32, name="abt3")
        nc.gpsimd.dma_start(abt3[:], ab3[:])
        layer_dr(w3_d, abt3, NT1, A2, A3, 3)
        abt4 = abt_pool.tile([P, NT4 * 2], dt.float32, name="abt4")
        nc.gpsimd.dma_start(abt4[:], ab4[:])
        layer_dr(w4_d, abt4, NT4, A3, None, 4)

        wf_cm.__exit__(None, None, None)
        apool23_cm.__exit__(None, None, None)
        apool12_cm.__exit__(None, None, None)
        tmp_cm.__exit__(None, None, None)
        abt_cm.__exit__(None, None, None)
        plpool_cm.__exit__(None, None, None)
        ppool_cm.__exit__(None, None, None)

    nc.compile()
    return nc


def _arr_w(Wt, NT):
    """[K, N] -> [NT, P, KT*P] with partition-contiguous per-nt blocks."""
    K = Wt.shape[0]
    kt = K // P
    # [K, N] -> (kt, P, NT, P) -> (NT, P_part, kt, P_n)
    a = Wt.reshape(kt, P, NT, P).transpose(2, 1, 0, 3).reshape(NT, P, kt * P)
    return np.ascontiguousarray(a)


def _host_prep(inputs):
    f32 = np.float32
    fp8 = ml_dtypes.float8_e4m3

    def levels(W):
        s = f32(np.max(np.abs(W))) / f32(3.0)
        return np.clip(np.round(W / s), -3.0, 3.0).astype(f32), s

    L1, sw1 = levels(inputs["W1"])
    L2, sw2 = levels(inputs["W2"])
    L3, sw3 = levels(inputs["W3"])
    L4, sw4 = levels(inputs["W4"])
    s_a = [f32(inputs[k][0]) for k in ("s1", "s2", "s3")]

    def fold(l, s_w, s_prev):
        g = inputs[f"g{l}"].astype(np.float64)
        be = inputs[f"be{l}"].astype(np.float64)
        m = inputs[f"m{l}"].astype(np.float64)
        v = inputs[f"v{l}"].astype(np.float64)
        b = inputs[f"b{l}"].astype(np.float64)
        inv = 1.0 / np.sqrt(v + EPS)
        sl = float(s_a[l - 1])
        alpha = (float(s_prev) * float(s_w) * g * inv) / sl
        beta = ((b - m) * inv * g + be) / sl
        return alpha.astype(f32), beta.astype(f32)

    a1, b1 = fold(1, sw1, 1.0)
    a2, b2 = fold(2, sw2, s_a[0])
    a3, b3 = fold(3, sw3, s_a[1])
    a4 = np.full(N4P, float(s_a[2]) * float(sw4), dtype=f32)
    b4 = np.zeros(N4P, dtype=f32)
    b4[:C_OUT] = inputs["b4"]

    def abpack(a, b, NT):
        # [N] alpha, [N] beta -> [P, NT*2] with (alpha, beta) interleaved
        ab = np.stack([a, b], axis=1).reshape(NT, P, 2)
        return np.ascontiguousarray(ab.transpose(1, 0, 2).reshape(P, NT * 2))

    w1h = _arr_w(L1.T, NT1).astype(np.float16)
    w1l = _arr_w(L1.T * f32(1.0 / LO_SC), NT1).astype(fp8)
    w2 = _arr_w(L2.T, NT1).astype(fp8)
    w3 = _arr_w(L3.T, NT1).astype(fp8)
    L4p = np.zeros((N4P, H), dtype=f32)
    L4p[:C_OUT] = L4
    w4 = _arr_w(L4p.T, NT4).astype(fp8)

    shared = dict(
        w1h=w1h, w1l=w1l, w2=w2, w3=w3, w4=w4,
        ab1=abpack(a1, b1, NT1), ab2=abpack(a2, b2, NT1),
        ab3=abpack(a3, b3, NT1), ab4=abpack(a4, b4, NT4),
    )
    xt = inputs["x"].T  # [D_IN, B] view
    in_maps = []
    for c in range(NCORES):
        xs = np.ascontiguousarray(xt[:, c * BC:(c + 1) * BC], dtype=f32)
        xh = xs.astype(np.float16)
        r = xs - xh.astype(f32)
        xl = (r * f32(LO_SC)).astype(fp8)
        # [D_IN, BC] -> [P, KTX*BC] partition-contiguous
        m = dict(shared)
        m["xh"] = np.ascontiguousarray(
            xh.reshape(KTX, P, BC).transpose(1, 0, 2).reshape(P, KTX * BC))
        m["xl"] = np.ascontiguousarray(
            xl.reshape(KTX, P, BC).transpose(1, 0, 2).reshape(P, KTX * BC))
        in_maps.append(m)
    return in_maps


def kernel(**inputs):
    from concourse.bass_utils import run_bass_kernel_spmd

    inputs = {k: np.asarray(v) for k, v in inputs.items()}
    if "nc" not in _CACHE:
        _CACHE["nc"] = _build_nc()
    nc = _CACHE["nc"]

    in_maps = _host_prep(inputs)
    res = run_bass_kernel_spmd(nc, in_maps, core_ids=list(range(NCORES)))

    out = np.empty((B, C_OUT), dtype=np.float32)
    for c in range(NCORES):
        out[c * BC:(c + 1) * BC, :] = res.results[c]["out_t"][:C_OUT, :].T
    return out
